# revision 1
# baseline (speedup 1.0000x reference)
"""Trainium2 Bass kernel for a 2-layer GAT (nn_DepthAwareGAT).

Self-contained: hardcodes problem shapes; accepts FULL inputs, returns FULL
output. Strategy:
  - dst-range sharding across 8 cores (12500 nodes each; edge counts balance
    to ~0.5% since dst is uniform).
  - Node tables TAB1/TAB2 hold per-node rows [h | raw-f32-bits of
    (e_src, e_dst)] in bf16; per-edge h[src] rows fetched with int16
    dma_gather (4 SWDGE queues, 4 src-range subtables).
  - Segment softmax+aggregate in ONE pass per layer via mask matmuls:
    out[d] = (sum_e exp(e_e) h[src_e]) / (sum_e exp(e_e)); no segment max
    needed (logits are O(1), exp cannot overflow).
  - e_dst per edge expanded on-chip: M = onehot(dst_local), MT = PE-transpose,
    e_dst_edge = MT @ e_dst_window.
  - h2 = elu(out1) kept on-chip; TAB2 shards AllGathered between layers.
"""

import hashlib

import numpy as np
import ml_dtypes

import concourse.bacc as bacc
import concourse.bass as bass
import concourse.tile as tile
import concourse.mybir as mybir
from concourse.bass_utils import run_bass_kernel_spmd

F32 = mybir.dt.float32
BF16 = mybir.dt.bfloat16
I16 = mybir.dt.int16
AF = mybir.ActivationFunctionType
OP = mybir.AluOpType


class Cfg:
    def __init__(self, N=100000, IN_DIM=128, HID=64, HEADS=4, OUT_DIM=32):
        self.N = N
        self.IN_DIM = IN_DIM
        self.HID = HID
        self.HEADS = HEADS
        self.OUT_DIM = OUT_DIM
        self.SLOPE = 0.2
        self.NCORES = 8
        self.WIN = 128
        self.NSUB = 4
        self.NI = 1024
        self.PADDL = 200.0
        self.RING = 2
        self.C1 = HID * HEADS                       # 256
        self.CE1 = 384                              # bf16 row: 768B (%256 ok)
        self.U1 = self.C1 + 16                      # used cols: h + 8 f32 bits
        self.CE2 = 128                              # 256B
        self.U2 = OUT_DIM + 4                       # h2 + 2 f32 bits
        self.R = N // self.NCORES
        self.NWIN = (self.R + self.WIN - 1) // self.WIN
        self.SUBN = N // self.NSUB
        self.KCALL = self.NI // 128
        self.NT1 = (N + 127) // 128


def _wrap_calls(flat):
    """[ncalls, NI] int -> [ncalls, 128, NI//16] int16 wrapped layout."""
    nc_, ni = flat.shape
    a = flat.reshape(nc_, ni // 16, 16).transpose(0, 2, 1)
    return np.tile(a, (1, 8, 1)).astype(np.int16)


def _build_meta(edge_index, cfg):
    """Host-side prep: per-core gather streams, mask metadata, schedule."""
    src = np.asarray(edge_index[0], np.int64)
    dst = np.asarray(edge_index[1], np.int64)
    R, WIN, NWIN, NSUB = cfg.R, cfg.WIN, cfg.NWIN, cfg.NSUB
    NI, KCALL = cfg.NI, cfg.KCALL

    core = dst // R
    w_all = (dst % R) // WIN
    q_all = src // cfg.SUBN
    counts = np.zeros((cfg.NCORES, NWIN, NSUB), np.int64)
    np.add.at(counts, (core, w_all, q_all), 1)
    kwq = (-(-counts // 128)).max(axis=0)         # [NWIN, NSUB]
    nsub_q = kwq.sum(axis=0)
    ncalls_q = -(-nsub_q // KCALL)
    ncalls_q = np.maximum(ncalls_q, 1)
    nsub_pad_q = ncalls_q * KCALL
    off = np.zeros((NWIN, NSUB), np.int64)
    off[1:] = np.cumsum(kwq, axis=0)[:-1]

    per_core = []
    for c in range(cfg.NCORES):
        sel = core == c
        s = src[sel]
        d = dst[sel] - c * R
        w = d // WIN
        dl = (d % WIN).astype(np.float32)
        q = s // cfg.SUBN
        sl = (s % cfg.SUBN).astype(np.int64)
        o = np.lexsort((sl, q, w))
        w, dl, q, sl = w[o], dl[o], q[o], sl[o]
        gidx, gdl = [], []
        for qq in range(NSUB):
            m = q == qq
            wq, dlq, slq = w[m], dl[m], sl[m]
            nslots = int(nsub_pad_q[qq]) * 128
            slots_i = np.zeros(nslots, np.int64)
            slots_d = np.full(nslots, cfg.PADDL, np.float32)
            grp_start = np.searchsorted(wq, np.arange(NWIN))
            pos = off[wq, qq] * 128 + (np.arange(len(wq)) - grp_start[wq])
            slots_i[pos] = slq
            slots_d[pos] = dlq
            ncalls = int(ncalls_q[qq])
            wrapped = _wrap_calls(slots_i.reshape(ncalls, NI))
            dlb = (slots_d.reshape(ncalls, KCALL, 128).transpose(0, 2, 1)
                   .astype(ml_dtypes.bfloat16))
            # pack [idx_wrapped | dl bf16 bits] into one int16 tensor
            comb = np.concatenate([wrapped, dlb.view(np.int16)], axis=2)
            gidx.append(comb)
            # transposed one-hot masks MT[d, e] per subtile, bf16
            dlv = slots_d.reshape(ncalls, KCALL, 128)      # [r, t, e]
            mt = (np.arange(128, dtype=np.float32)[None, None, :, None]
                  == dlv[:, :, None, :])                   # [r, t, d, e]
            mt = (mt.transpose(0, 2, 1, 3)                 # [r, d, t, e]
                  .reshape(ncalls, 128, KCALL * 128)
                  .astype(ml_dtypes.bfloat16))
            gdl.append(mt)
        per_core.append({"gidx": gidx, "gdl": gdl})

    # window of each stream subtile (tail dummies -> last window)
    win_of_sub = []
    for qq in range(NSUB):
        lst = []
        for w in range(NWIN):
            lst.extend([w] * int(kwq[w, qq]))
        lst.extend([NWIN - 1] * int(nsub_pad_q[qq] - nsub_q[qq]))
        win_of_sub.append(lst)
    return {"per_core": per_core, "win_of_sub": win_of_sub,
            "ncalls_q": [int(x) for x in ncalls_q]}


def _edge_pass(nc, cfg, tc, consts, meta, layer, subtabs, own, gidx_dr,
               gmt_dr, h2sb, emb, b2rep):
    """One edge-parallel layer pass, call-major with per-call batched ops."""
    R, WIN, NWIN, NSUB = cfg.R, cfg.WIN, cfg.NWIN, cfg.NSUB
    NI, K = cfg.NI, cfg.KCALL
    CE = cfg.CE1 if layer == 1 else cfg.CE2
    U = cfg.U1 if layer == 1 else cfg.U2
    NCH = cfg.C1 if layer == 1 else cfg.OUT_DIM
    NH = cfg.HEADS if layer == 1 else 1
    CW = NCH // NH
    NACC = NCH + NH
    iota, ident_bf = consts["iota"], consts["ident_bf"]
    win_of_sub = meta["win_of_sub"]
    ncalls_q = meta["ncalls_q"]
    NIW = NI // 16

    remaining = [0] * NWIN
    for qq in range(NSUB):
        for w in win_of_sub[qq]:
            remaining[w] += 1

    with tc.tile_pool(name=f"ring{layer}", bufs=cfg.RING) as ringp, \
         tc.tile_pool(name=f"meta{layer}", bufs=cfg.RING + 2) as metap, \
         tc.tile_pool(name=f"mt{layer}", bufs=3) as mtpool, \
         tc.tile_pool(name=f"work{layer}", bufs=4) as wp, \
         tc.tile_pool(name=f"mwork{layer}", bufs=3) as mwp, \
         tc.tile_pool(name=f"blk{layer}", bufs=4) as blkp, \
         tc.tile_pool(name=f"accps{layer}", bufs=5, space="PSUM") as accp, \
         tc.tile_pool(name=f"mtps{layer}", bufs=2, space="PSUM") as mtpp, \
         tc.tile_pool(name=f"edps{layer}", bufs=1, space="PSUM") as edpp:

        blk_tiles, acc_tiles, edstb_tiles = {}, {}, {}

        def open_window(w):
            dn = min(WIN, R - w * WIN)
            blk = blkp.tile([128, U], BF16, tag="blk")
            if dn < 128:
                nc.gpsimd.memset(blk[:], 0.0)
            nc.sync.dma_start(out=blk[:dn, :],
                              in_=own[w * WIN:w * WIN + dn, 0:U])
            acc = accp.tile([128, NACC], F32, tag="acc", space="PSUM")
            blk_tiles[w] = blk
            acc_tiles[w] = acc
            esrc_self = blk[:, NCH:NCH + 2 * NH].bitcast(F32)
            edst_w = blk[:, NCH + 2 * NH:NCH + 4 * NH].bitcast(F32)
            edstb = blkp.tile([128, NH], BF16, tag="edstb")
            nc.vector.tensor_copy(out=edstb[:], in_=edst_w)
            edstb_tiles[w] = edstb
            lg = wp.tile([128, NH], F32, tag="lg0")
            nc.vector.tensor_tensor(out=lg[:], in0=esrc_self, in1=edst_w,
                                    op=OP.add)
            lr = wp.tile([128, NH], F32, tag="lr0")
            nc.scalar.activation(out=lr[:], in_=lg[:], func=AF.Prelu,
                                 alpha=cfg.SLOPE)
            ex = wp.tile([128, NH], F32, tag="ex0")
            nc.scalar.activation(out=ex[:], in_=lr[:], func=AF.Exp)
            nc.vector.tensor_copy(out=blk[:, NCH:NCH + NH], in_=ex[:])
            den3 = blk[:, NCH:NCH + NH].rearrange(
                "p (h o) -> p h o", o=1).broadcast_to([128, NH, CW])
            b3 = blk[:, 0:NCH].rearrange("p (h c) -> p h c", c=CW)
            nc.vector.tensor_tensor(out=b3, in0=b3, in1=den3, op=OP.mult)
            nc.tensor.matmul(out=acc[:], lhsT=ident_bf[:],
                             rhs=blk[:, 0:NACC], start=True,
                             stop=(remaining[w] == 0))

        def close_window(w):
            acc = acc_tiles.pop(w)
            blk_tiles.pop(w)
            edstb_tiles.pop(w)
            dn = min(WIN, R - w * WIN)
            recip = wp.tile([128, NH], F32, tag="recip")
            nc.vector.reciprocal(out=recip[:], in_=acc[:, NCH:NCH + NH])
            rb = recip[:].rearrange("p (h o) -> p h o", o=1) \
                         .broadcast_to([128, NH, CW])
            if layer == 1:
                hw = h2sb[:, w * NCH:(w + 1) * NCH]
                hw3 = hw.rearrange("p (h c) -> p h c", c=CW)
                ac3 = acc[:, 0:NCH].rearrange("p (h c) -> p h c", c=CW)
                nc.vector.tensor_tensor(out=hw3, in0=ac3, in1=rb, op=OP.mult)
                if consts.get("b1rep") is not None:
                    nc.vector.tensor_tensor(out=hw, in0=hw,
                                            in1=consts["b1rep"][:], op=OP.add)
                # ELU: (max(x,0)-1) + exp(-relu(-x))
                t1 = wp.tile([128, NCH], F32, tag="elu1")
                nc.scalar.activation(out=t1[:], in_=hw, func=AF.Relu,
                                     scale=-1.0)
                nc.scalar.activation(out=t1[:], in_=t1[:], func=AF.Exp,
                                     scale=-1.0)
                nc.vector.tensor_scalar(out=hw, in0=hw, scalar1=0.0,
                                        scalar2=-1.0, op0=OP.max, op1=OP.add)
                nc.vector.tensor_tensor(out=hw, in0=hw, in1=t1[:], op=OP.add)
            else:
                ot = wp.tile([128, cfg.OUT_DIM], F32, tag="ot")
                nc.vector.tensor_scalar_mul(out=ot[:],
                                            in0=acc[:, 0:cfg.OUT_DIM],
                                            scalar1=recip[:, 0:1])
                nc.vector.tensor_tensor(out=ot[:], in0=ot[:], in1=b2rep[:],
                                        op=OP.add)
                nc.scalar.dma_start(out=emb[w * WIN:w * WIN + dn, :],
                                    in_=ot[:dn, :])

        rounds = max(ncalls_q)
        for r in range(rounds):
            for qq in range(NSUB):
                if r >= ncalls_q[qq]:
                    continue
                idxt = metap.tile([128, NIW + K], I16, tag=f"i{qq}")
                nc.sync.dma_start(out=idxt[:], in_=gidx_dr[qq][r, :, :])
                dlt = idxt[:, NIW:NIW + K].bitcast(BF16)
                mtt = mtpool.tile([128, K * 128], BF16, tag=f"mt{qq}")
                nc.scalar.dma_start(out=mtt[:], in_=gmt_dr[qq][r, :, :])
                gt = ringp.tile([128, K * CE], BF16, tag=f"g{qq}")
                nc.gpsimd.dma_gather(
                    gt[:].rearrange("p (k c) -> p k c", c=CE),
                    subtabs[qq], idxt[:, 0:NIW], NI, NI, CE, queue_num=qq,
                )
                subw = [win_of_sub[qq][r * K + t] for t in range(K)]
                for w in sorted(set(subw)):
                    if w not in acc_tiles:
                        open_window(w)

                g3 = gt[:].rearrange("p (k c) -> p k c", c=CE)
                mtp = mtpp.tile([128, K * 128], BF16, tag="mtp", space="PSUM")
                for t in range(K):
                    nc.tensor.transpose(out=mtp[:, t * 128:(t + 1) * 128],
                                        in_=mtt[:, t * 128:(t + 1) * 128],
                                        identity=ident_bf[:])
                mall = mwp.tile([128, K * 128], BF16, tag="mall")
                nc.scalar.activation(out=mall[:], in_=mtp[:], func=AF.Copy)
                edp = edpp.tile([128, K * NH], F32, tag="edp", space="PSUM")
                for t in range(K):
                    nc.tensor.matmul(
                        out=edp[:, t * NH:(t + 1) * NH],
                        lhsT=mtt[:, t * 128:(t + 1) * 128],
                        rhs=edstb_tiles[subw[t]][:],
                        start=True, stop=True)
                lg = wp.tile([128, K * NH], F32, tag="lg")
                nc.vector.tensor_tensor(
                    out=lg[:].rearrange("p (k h) -> p k h", h=NH),
                    in0=g3[:, :, NCH:NCH + 2 * NH].bitcast(F32),
                    in1=edp[:].rearrange("p (k h) -> p k h", h=NH),
                    op=OP.add)
                lr = wp.tile([128, K * NH], F32, tag="lr")
                nc.scalar.activation(out=lr[:], in_=lg[:], func=AF.Prelu,
                                     alpha=cfg.SLOPE)
                ex = wp.tile([128, K * NH], F32, tag="ex")
                nc.scalar.activation(out=ex[:], in_=lr[:], func=AF.Exp)
                nc.vector.tensor_copy(
                    out=g3[:, :, NCH:NCH + NH],
                    in_=ex[:].rearrange("p (k h) -> p k h", h=NH))
                den4 = g3[:, :, NCH:NCH + NH].rearrange(
                    "p k (h o) -> p k h o", o=1).broadcast_to([128, K, NH, CW])
                g4 = g3[:, :, 0:NCH].rearrange("p k (h c) -> p k h c", c=CW)
                nc.vector.tensor_tensor(out=g4, in0=g4, in1=den4, op=OP.mult)
                for t in range(K):
                    w = subw[t]
                    nc.tensor.matmul(out=acc_tiles[w][:],
                                     lhsT=mall[:, t * 128:(t + 1) * 128],
                                     rhs=gt[:, t * CE:t * CE + NACC],
                                     start=False,
                                     stop=(remaining[w] == 1))
                    remaining[w] -= 1
                    if remaining[w] == 0:
                        close_window(w)


def _build_bass(meta, cfg, b1_nonzero, phases=4):
    nc = bacc.Bacc("TRN2", target_bir_lowering=False, debug=False,
                   num_devices=cfg.NCORES, num_swdge_queues=4)
    N, R, NWIN, NSUB, C1 = cfg.N, cfg.R, cfg.NWIN, cfg.NSUB, cfg.C1

    x = nc.dram_tensor("x", [N, cfg.IN_DIM], F32, kind="ExternalInput")
    w1x = nc.dram_tensor("w1x", [cfg.IN_DIM, C1 + 8], F32,
                         kind="ExternalInput")
    w2x = nc.dram_tensor("w2x", [128, 2 * (cfg.OUT_DIM + 2)], F32,
                         kind="ExternalInput")
    iota_in = nc.dram_tensor("iota", [128, cfg.KCALL * 128], BF16,
                             kind="ExternalInput")
    identb_in = nc.dram_tensor("identb", [128, 128], BF16,
                               kind="ExternalInput")
    identf_in = nc.dram_tensor("identf", [128, 128], F32,
                               kind="ExternalInput")
    b2rep_in = nc.dram_tensor("b2rep", [128, cfg.OUT_DIM], F32,
                              kind="ExternalInput")
    if b1_nonzero:
        b1rep_in = nc.dram_tensor("b1rep", [128, C1], F32,
                                  kind="ExternalInput")
    gidx_dr, gdl_dr = [], []
    for qq in range(NSUB):
        ncalls = meta["ncalls_q"][qq]
        gidx_dr.append(nc.dram_tensor(
            f"gidx{qq}", [ncalls, 128, cfg.NI // 16 + cfg.KCALL], I16,
            kind="ExternalInput"))
        gdl_dr.append(nc.dram_tensor(
            f"gdl{qq}", [ncalls, 128, cfg.KCALL * 128], BF16,
            kind="ExternalInput"))
    emb = nc.dram_tensor("emb", [R, cfg.OUT_DIM], F32, kind="ExternalOutput")

    with tile.TileContext(nc) as tc:
        with tc.tile_pool(name="const", bufs=1) as cp, \
             tc.tile_pool(name="dram", bufs=1, space="DRAM") as dp, \
             tc.tile_pool(name="h2p", bufs=1) as h2p:
            consts = {}
            iota = cp.tile([128, cfg.KCALL * 128], BF16)
            nc.sync.dma_start(out=iota[:], in_=iota_in[:])
            ident_bf = cp.tile([128, 128], BF16)
            nc.sync.dma_start(out=ident_bf[:], in_=identb_in[:])
            ident_f = cp.tile([128, 128], F32)
            nc.sync.dma_start(out=ident_f[:], in_=identf_in[:])
            w1x_sb = cp.tile([128, C1 + 8], F32)
            nc.sync.dma_start(out=w1x_sb[:], in_=w1x[:])
            w2x_sb = cp.tile([128, 2 * (cfg.OUT_DIM + 2)], F32)
            nc.sync.dma_start(out=w2x_sb[:], in_=w2x[:])
            b2rep = cp.tile([128, cfg.OUT_DIM], F32)
            nc.sync.dma_start(out=b2rep[:], in_=b2rep_in[:])
            consts["iota"] = iota
            consts["ident_bf"] = ident_bf
            if b1_nonzero:
                b1rep = cp.tile([128, C1], F32)
                nc.sync.dma_start(out=b1rep[:], in_=b1rep_in[:])
                consts["b1rep"] = b1rep
            else:
                consts["b1rep"] = None

            tab1 = dp.tile([N, cfg.CE1], BF16)
            tab2 = dp.tile([N, cfg.CE2], BF16)
            ag_in = dp.tile([R, cfg.CE2], BF16)
            own1 = dp.tile([R, cfg.U1], BF16)
            own2 = dp.tile([R, cfg.U2], BF16)
            h2sb = h2p.tile([128, NWIN * C1], F32)

            # ---- phase 1: TAB1[n] = [bf16(x@W1), f32 bits of (esrc,edst)] ----
            with tc.tile_pool(name="p1", bufs=4) as p1, \
                 tc.tile_pool(name="p1ps", bufs=2, space="PSUM") as p1ps, \
                 tc.tile_pool(name="p1xp", bufs=3, space="PSUM") as p1xp:
                NMAC = N // 256
                for t2 in range(NMAC):
                    r0 = t2 * 256
                    xt = p1.tile([128, 256], F32, tag="xt")
                    nc.sync.dma_start(
                        out=xt[:].rearrange("p (a c) -> p a c", a=2),
                        in_=x[r0:r0 + 256, :].rearrange("(a p) c -> p a c",
                                                        p=128))
                    hp = p1ps.tile([128, 1024], F32, tag="hp", space="PSUM")
                    for a in range(2):
                        xp = p1xp.tile([128, 128], F32, tag="xp", space="PSUM")
                        nc.tensor.transpose(out=xp[:],
                                            in_=xt[:, a * 128:(a + 1) * 128],
                                            identity=ident_f[:])
                        xT = p1.tile([128, 128], F32, tag="xT")
                        nc.scalar.activation(out=xT[:], in_=xp[:],
                                             func=AF.Copy)
                        nc.tensor.matmul(out=hp[:, a * 512:a * 512 + C1 + 8],
                                         lhsT=xT[:], rhs=w1x_sb[:],
                                         start=True, stop=True)
                    tb = p1.tile([128, 2 * cfg.U1], BF16, tag="tb")
                    tb3 = tb[:].rearrange("p (a u) -> p a u", a=2)
                    hp3 = hp[:].rearrange("p (a u) -> p a u", a=2)
                    nc.scalar.activation(out=tb3[:, :, 0:C1],
                                         in_=hp3[:, :, 0:C1], func=AF.Copy)
                    nc.vector.tensor_copy(
                        out=tb3[:, :, C1:C1 + 16].bitcast(F32),
                        in_=hp3[:, :, C1:C1 + 8])
                    nc.scalar.dma_start(
                        out=tab1[r0:r0 + 256, 0:cfg.U1].rearrange(
                            "(a p) u -> p a u", p=128),
                        in_=tb3)
                for t in range(NMAC * 2, cfg.NT1):
                    dn = min(128, N - t * 128)
                    xt = p1.tile([128, cfg.IN_DIM], F32, tag="xts")
                    if dn < 128:
                        nc.gpsimd.memset(xt[:], 0.0)
                    nc.sync.dma_start(out=xt[:dn, :],
                                      in_=x[t * 128:t * 128 + dn, :])
                    xp = p1xp.tile([128, 128], F32, tag="xp", space="PSUM")
                    nc.tensor.transpose(out=xp[:], in_=xt[:],
                                        identity=ident_f[:])
                    xT = p1.tile([128, 128], F32, tag="xT")
                    nc.scalar.activation(out=xT[:], in_=xp[:], func=AF.Copy)
                    hp = p1ps.tile([128, 1024], F32, tag="hp", space="PSUM")
                    nc.tensor.matmul(out=hp[:, 0:C1 + 8], lhsT=xT[:],
                                     rhs=w1x_sb[:], start=True, stop=True)
                    tb = p1.tile([128, cfg.U1], BF16, tag="tbs")
                    nc.vector.tensor_copy(out=tb[:, 0:C1], in_=hp[:, 0:C1])
                    nc.vector.tensor_copy(out=tb[:, C1:C1 + 16].bitcast(F32),
                                          in_=hp[:, C1:C1 + 8])
                    nc.sync.dma_start(out=tab1[t * 128:t * 128 + dn, 0:cfg.U1],
                                      in_=tb[:dn, :])

            # own1 = tab1 rows of this core's dst range (dynamic on core id)
            rid = nc.sync.partition_id()
            tab1_3d = tab1[:, 0:cfg.U1].rearrange("(c r) u -> c r u",
                                                  c=cfg.NCORES)
            nc.sync.dma_start(out=own1[:, :].rearrange("(o r) u -> o r u", o=1),
                              in_=tab1_3d[bass.ds(rid, 1), :, :])

            if phases == 1:
                dmy = cp.tile([128, cfg.OUT_DIM], BF16)
                nc.sync.dma_start(out=dmy[:], in_=own1[0:128, 0:cfg.OUT_DIM])
                nc.gpsimd.dma_start(out=emb[0:128, :], in_=dmy[:])
            if phases >= 2:
                # ---- phase 2: layer-1 edge pass -> h2sb ----
                sub1 = [tab1[qq * cfg.SUBN:(qq + 1) * cfg.SUBN, :]
                        for qq in range(NSUB)]
                _edge_pass(nc, cfg, tc, consts, meta, 1, sub1, own1, gidx_dr,
                           gdl_dr, h2sb, emb, b2rep)

            if phases == 2:
                dmy = cp.tile([128, cfg.OUT_DIM], F32)
                nc.vector.tensor_copy(out=dmy[:], in_=h2sb[:, 0:cfg.OUT_DIM])
                nc.sync.dma_start(out=emb[0:128, :], in_=dmy[:])
            # ---- phase 3: TAB2 shard = [bf16(h2@W2x) | f32 bits] + AllGather
            if phases >= 3:
              with tc.tile_pool(name="p4", bufs=4) as p4, \
                 tc.tile_pool(name="p4ps", bufs=3, space="PSUM") as p4ps:
                for t in range(NWIN):
                    dn = min(128, R - t * 128)
                    h2Ts = []
                    for k in range(2):
                        hp2 = p4ps.tile([128, 128], F32, tag="tps",
                                        space="PSUM")
                        nc.tensor.transpose(
                            out=hp2[:],
                            in_=h2sb[:, t * C1 + k * 128:t * C1 + (k + 1) * 128],
                            identity=ident_f[:])
                        h2T = p4.tile([128, 128], F32, tag=f"h2T{k}")
                        nc.scalar.activation(out=h2T[:], in_=hp2[:],
                                             func=AF.Copy)
                        h2Ts.append(h2T)
                    W = cfg.OUT_DIM + 2
                    hh = p4ps.tile([128, W], F32, tag="hh", space="PSUM")
                    nc.tensor.matmul(out=hh[:], lhsT=h2Ts[0][:],
                                     rhs=w2x_sb[:, 0:W], start=True,
                                     stop=False)
                    nc.tensor.matmul(out=hh[:], lhsT=h2Ts[1][:],
                                     rhs=w2x_sb[:, W:2 * W], start=False,
                                     stop=True)
                    t2b = p4.tile([128, cfg.U2], BF16, tag="t2b")
                    nc.vector.tensor_copy(out=t2b[:, 0:cfg.OUT_DIM],
                                          in_=hh[:, 0:cfg.OUT_DIM])
                    nc.vector.tensor_copy(
                        out=t2b[:, cfg.OUT_DIM:cfg.OUT_DIM + 4].bitcast(F32),
                        in_=hh[:, cfg.OUT_DIM:cfg.OUT_DIM + 2])
                    nc.scalar.dma_start(
                        out=ag_in[t * 128:t * 128 + dn, 0:cfg.U2],
                        in_=t2b[:dn, :])
            if phases >= 3:
                nc.gpsimd.collective_compute(
                    "AllGather", OP.bypass,
                    replica_groups=[list(range(cfg.NCORES))],
                    ins=[ag_in[:, :]], outs=[tab2[:, :]],
                )
                own2_3d = tab2[:, 0:cfg.U2].rearrange("(c r) u -> c r u",
                                                      c=cfg.NCORES)
                nc.sync.dma_start(
                    out=own2[:, :].rearrange("(o r) u -> o r u", o=1),
                    in_=own2_3d[bass.ds(rid, 1), :, :])
            if phases == 3:
                dmy = cp.tile([128, cfg.OUT_DIM], BF16)
                nc.sync.dma_start(out=dmy[:], in_=own2[0:128, 0:cfg.OUT_DIM])
                nc.gpsimd.dma_start(out=emb[0:128, :], in_=dmy[:])
            if phases >= 4:
                # ---- phase 4: layer-2 edge pass -> emb ----
                sub2 = [tab2[qq * cfg.SUBN:(qq + 1) * cfg.SUBN, :]
                        for qq in range(NSUB)]
                _edge_pass(nc, cfg, tc, consts, meta, 2, sub2, own2, gidx_dr,
                           gdl_dr, h2sb, emb, b2rep)

    nc.compile()
    return nc


def _host_inputs(meta, cfg, x, W1, a_src1, a_dst1, b1, W2, a_src2, a_dst2, b2):
    C1 = cfg.C1
    w1e = np.zeros((cfg.IN_DIM, 8), np.float32)
    for j in range(cfg.HEADS):
        blkw = W1[:, j * cfg.HID:(j + 1) * cfg.HID]
        w1e[:, j] = blkw @ a_src1[j]
        w1e[:, 4 + j] = blkw @ a_dst1[j]
    w1x = np.concatenate([W1, w1e], axis=1).astype(np.float32)
    W = cfg.OUT_DIM + 2
    w2full = np.concatenate(
        [W2, (W2 @ a_src2[0])[:, None], (W2 @ a_dst2[0])[:, None]],
        axis=1).astype(np.float32)                      # [256, 34]
    w2x = np.concatenate([w2full[:128], w2full[128:]], axis=1)  # [128, 68]

    iota = np.tile(np.arange(128, dtype=np.float32),
                   (128, cfg.KCALL)).astype(ml_dtypes.bfloat16)
    identb = np.eye(128, dtype=np.float32).astype(ml_dtypes.bfloat16)
    identf = np.eye(128, dtype=np.float32)
    b2rep = np.tile(np.asarray(b2, np.float32)[None, :], (128, 1))
    base = {"x": np.asarray(x, np.float32), "w1x": w1x, "w2x": w2x,
            "iota": iota, "identb": identb, "identf": identf, "b2rep": b2rep}
    if np.any(b1):
        base["b1rep"] = np.tile(np.asarray(b1, np.float32)[None, :], (128, 1))
    in_maps = []
    for c in range(cfg.NCORES):
        m = dict(base)
        pc = meta["per_core"][c]
        for qq in range(cfg.NSUB):
            m[f"gidx{qq}"] = pc["gidx"][qq]
            m[f"gdl{qq}"] = pc["gdl"][qq]
        in_maps.append(m)
    return in_maps


_CACHE = {}


def _get_compiled(edge_index, cfg, b1_nonzero):
    key = (hashlib.sha1(np.ascontiguousarray(edge_index).tobytes())
           .hexdigest(), b1_nonzero, cfg.N)
    if key not in _CACHE:
        meta = _build_meta(edge_index, cfg)
        nc = _build_bass(meta, cfg, b1_nonzero)
        _CACHE[key] = (meta, nc)
    return _CACHE[key]


def kernel(x, edge_index, W1, a_src1, a_dst1, b1, W2, a_src2, a_dst2, b2):
    cfg = Cfg()
    x = np.asarray(x)
    edge_index = np.asarray(edge_index)
    # append the reference's self-loops? no: self-loops handled structurally,
    # but coincidental src==dst edges in edge_index flow the normal path.
    meta, nc = _get_compiled(edge_index, cfg, bool(np.any(np.asarray(b1))))
    in_maps = _host_inputs(meta, cfg, x, np.asarray(W1), np.asarray(a_src1),
                           np.asarray(a_dst1), np.asarray(b1), np.asarray(W2),
                           np.asarray(a_src2), np.asarray(a_dst2),
                           np.asarray(b2))
    res = run_bass_kernel_spmd(nc, in_maps, list(range(cfg.NCORES)))
    shards = [res.results[c]["emb"] for c in range(cfg.NCORES)]
    return np.concatenate(shards, axis=0)



# revision 5
# speedup vs baseline: 20.1395x; 20.1395x over previous
"""Trainium2 Bass kernel for a 2-layer GAT (nn_DepthAwareGAT).

Self-contained: hardcodes problem shapes; accepts FULL inputs, returns FULL
output. Strategy:
  - dst-range sharding across 8 cores (12500 nodes each; edge counts balance
    to ~0.5% since dst is uniform).
  - Node tables TAB1/TAB2 hold per-node rows [h | raw-f32-bits of
    (e_src, e_dst)] in bf16; per-edge h[src] rows fetched with int16
    dma_gather (4 SWDGE queues, 4 src-range subtables).
  - Segment softmax+aggregate in ONE pass per layer via mask matmuls:
    out[d] = (sum_e exp(e_e) h[src_e]) / (sum_e exp(e_e)); no segment max
    needed (logits are O(1), exp cannot overflow).
  - e_dst per edge expanded on-chip: MT = host-shipped onehot(dst_local)^T,
    e_dst_edge = MT @ e_dst_window; M = PE-transpose of MT feeds the
    accumulate matmuls.
  - Fused rounds: the 4 subtable streams share one index DMA, one mask DMA,
    one gather tile and single wide vector/scalar ops per round (4096 edges)
    to amortize per-op overheads.
  - h2 = elu(out1) kept on-chip (bf16); TAB2 shards AllGathered between
    layers; windows read TAB1/TAB2 directly at a dynamic core-id offset.
"""

import hashlib

import numpy as np
import ml_dtypes

import concourse.bacc as bacc
import concourse.bass as bass
import concourse.tile as tile
import concourse.mybir as mybir
from concourse.bass_utils import run_bass_kernel_spmd

F32 = mybir.dt.float32
BF16 = mybir.dt.bfloat16
I16 = mybir.dt.int16
AF = mybir.ActivationFunctionType
OP = mybir.AluOpType


class Cfg:
    def __init__(self, N=100000, IN_DIM=128, HID=64, HEADS=4, OUT_DIM=32):
        self.N = N
        self.IN_DIM = IN_DIM
        self.HID = HID
        self.HEADS = HEADS
        self.OUT_DIM = OUT_DIM
        self.SLOPE = 0.2
        self.NCORES = 8
        self.WIN = 128
        self.NSUB = 4
        self.NI = 1024
        self.PADDL = 200.0
        self.RING = 3
        self.C1 = HID * HEADS                       # 256
        self.CE1 = 384                              # bf16 row: 768B (%256 ok)
        self.U1 = self.C1 + 16                      # used cols: h + 8 f32 bits
        self.CE2 = 128                              # 256B
        self.U2 = OUT_DIM + 4                       # h2 + 2 f32 bits
        self.R = N // self.NCORES
        self.NWIN = (self.R + self.WIN - 1) // self.WIN
        self.SUBN = N // self.NSUB
        self.KCALL = self.NI // 128
        self.NT1 = (N + 127) // 128


def _wrap_calls(flat):
    """[ncalls, NI] int -> [ncalls, 128, NI//16] int16 wrapped layout."""
    nc_, ni = flat.shape
    a = flat.reshape(nc_, ni // 16, 16).transpose(0, 2, 1)
    return np.tile(a, (1, 8, 1)).astype(np.int16)


def _build_meta(edge_index, cfg):
    """Host-side prep: per-core fused round-major gather streams + masks."""
    src = np.asarray(edge_index[0], np.int64)
    dst = np.asarray(edge_index[1], np.int64)
    R, WIN, NWIN, NSUB = cfg.R, cfg.WIN, cfg.NWIN, cfg.NSUB
    NI, KCALL = cfg.NI, cfg.KCALL
    NIW = NI // 16

    core = dst // R
    w_all = (dst % R) // WIN
    q_all = src // cfg.SUBN
    counts = np.zeros((cfg.NCORES, NWIN, NSUB), np.int64)
    np.add.at(counts, (core, w_all, q_all), 1)
    kwq = (-(-counts // 128)).max(axis=0)         # [NWIN, NSUB]
    nsub_q = kwq.sum(axis=0)
    ncalls_q = -(-nsub_q // KCALL)
    ncalls_q = np.maximum(ncalls_q, 1)
    rmax = int(ncalls_q.max())
    nsub_pad_q = np.full(NSUB, rmax * KCALL, np.int64)
    off = np.zeros((NWIN, NSUB), np.int64)
    off[1:] = np.cumsum(kwq, axis=0)[:-1]

    per_core = []
    for c in range(cfg.NCORES):
        sel = core == c
        s = src[sel]
        d = dst[sel] - c * R
        w = d // WIN
        dl = (d % WIN).astype(np.float32)
        q = s // cfg.SUBN
        sl = (s % cfg.SUBN).astype(np.int64)
        o = np.lexsort((sl, q, w))
        w, dl, q, sl = w[o], dl[o], q[o], sl[o]
        gidx_q, gmt_q = [], []
        for qq in range(NSUB):
            m = q == qq
            wq, dlq, slq = w[m], dl[m], sl[m]
            nslots = int(nsub_pad_q[qq]) * 128
            slots_i = np.zeros(nslots, np.int64)
            slots_d = np.full(nslots, cfg.PADDL, np.float32)
            grp_start = np.searchsorted(wq, np.arange(NWIN))
            pos = off[wq, qq] * 128 + (np.arange(len(wq)) - grp_start[wq])
            slots_i[pos] = slq
            slots_d[pos] = dlq
            wrapped = _wrap_calls(slots_i.reshape(rmax, NI))
            dlb = (slots_d.reshape(rmax, KCALL, 128).transpose(0, 2, 1)
                   .astype(ml_dtypes.bfloat16))
            gidx_q.append(np.concatenate([wrapped, dlb.view(np.int16)],
                                         axis=2))
            # transposed one-hot masks MT[d, e] per subtile, bf16
            dlv = slots_d.reshape(rmax, KCALL, 128)      # [r, t, e]
            mt = (np.arange(128, dtype=np.float32)[None, None, :, None]
                  == dlv[:, :, None, :])                 # [r, t, d, e]
            mt = (mt.transpose(0, 2, 1, 3)               # [r, d, t, e]
                  .reshape(rmax, 128, KCALL * 128)
                  .astype(ml_dtypes.bfloat16))
            gmt_q.append(mt)
        gidxf = np.concatenate(gidx_q, axis=2)           # [r, 128, 4*(NIW+K)]
        gmtf = np.concatenate(gmt_q, axis=2)             # [r, 128, 4*K*128]
        per_core.append({"gidxf": np.ascontiguousarray(gidxf),
                         "gmtf": np.ascontiguousarray(gmtf)})

    # window of each (stream, subtile); tail dummies -> last window
    win_of_sub = []
    for qq in range(NSUB):
        lst = []
        for w in range(NWIN):
            lst.extend([w] * int(kwq[w, qq]))
        lst.extend([NWIN - 1] * int(nsub_pad_q[qq] - len(lst)))
        win_of_sub.append(lst)

    # per-round window list across streams (gt-column order), plus max
    # concurrently-open windows for PSUM sizing
    subw_rounds = []
    for r in range(rmax):
        subw_rounds.append([win_of_sub[qq][r * KCALL + t]
                            for qq in range(NSUB) for t in range(KCALL)])
    remaining = np.zeros(NWIN, np.int64)
    for row in subw_rounds:
        for w in row:
            remaining[w] += 1
    open_set, max_open = set(), 0
    rem = remaining.copy()
    for row in subw_rounds:
        for w in set(row):
            open_set.add(w)
        max_open = max(max_open, len(open_set))
        for w in row:
            rem[w] -= 1
            if rem[w] == 0:
                open_set.discard(w)
    return {"per_core": per_core, "subw_rounds": subw_rounds,
            "rmax": rmax, "max_open": max_open}


def _edge_pass(nc, cfg, tc, consts, meta, layer, subtabs, own, gidx_dr,
               gmt_dr, h2sb, emb, b2rep):
    """One edge-parallel layer pass; fused rounds over the 4 streams."""
    R, WIN, NWIN, NSUB = cfg.R, cfg.WIN, cfg.NWIN, cfg.NSUB
    NI, K = cfg.NI, cfg.KCALL
    KF = NSUB * K                                   # subtiles per round
    CE = cfg.CE1 if layer == 1 else cfg.CE2
    U = cfg.U1 if layer == 1 else cfg.U2
    NCH = cfg.C1 if layer == 1 else cfg.OUT_DIM
    NH = cfg.HEADS if layer == 1 else 1
    CW = NCH // NH
    NACC = NCH + NH
    ident_bf = consts["ident_bf"]
    subw_rounds = meta["subw_rounds"]
    rmax = meta["rmax"]
    NIW = NI // 16
    SEG = NIW + K                                   # idxt cols per stream

    remaining = [0] * NWIN
    for row in subw_rounds:
        for w in row:
            remaining[w] += 1

    accbufs = max(5, meta["max_open"] + 1)
    with tc.tile_pool(name=f"ring{layer}", bufs=cfg.RING) as ringp, \
         tc.tile_pool(name=f"meta{layer}", bufs=3) as metap, \
         tc.tile_pool(name=f"mt{layer}", bufs=2) as mtpool, \
         tc.tile_pool(name=f"work{layer}", bufs=4) as wp, \
         tc.tile_pool(name=f"mwork{layer}", bufs=2) as mwp, \
         tc.tile_pool(name=f"blk{layer}", bufs=4) as blkp, \
         tc.tile_pool(name=f"accps{layer}", bufs=accbufs,
                      space="PSUM") as accp, \
         tc.tile_pool(name=f"mtps{layer}", bufs=2, space="PSUM") as mtpp, \
         tc.tile_pool(name=f"edps{layer}", bufs=1, space="PSUM") as edpp:

        blk_tiles, acc_tiles, edstb_tiles = {}, {}, {}

        def open_window(w):
            dn = min(WIN, R - w * WIN)
            blk = blkp.tile([128, U], BF16, tag="blk")
            if dn < 128:
                nc.gpsimd.memset(blk[:], 0.0)
            nc.sync.dma_start(out=blk[:dn, :],
                              in_=own[w * WIN:w * WIN + dn, 0:U])
            acc = accp.tile([128, NACC], F32, tag="acc", space="PSUM")
            blk_tiles[w] = blk
            acc_tiles[w] = acc
            esrc_self = blk[:, NCH:NCH + 2 * NH].bitcast(F32)
            edst_w = blk[:, NCH + 2 * NH:NCH + 4 * NH].bitcast(F32)
            edstb = blkp.tile([128, NH], BF16, tag="edstb")
            nc.vector.tensor_copy(out=edstb[:], in_=edst_w)
            edstb_tiles[w] = edstb
            lg = wp.tile([128, NH], F32, tag="lg0")
            nc.vector.tensor_tensor(out=lg[:], in0=esrc_self, in1=edst_w,
                                    op=OP.add)
            lr = wp.tile([128, NH], F32, tag="lr0")
            nc.scalar.activation(out=lr[:], in_=lg[:], func=AF.Prelu,
                                 alpha=cfg.SLOPE)
            ex = wp.tile([128, NH], F32, tag="ex0")
            nc.scalar.activation(out=ex[:], in_=lr[:], func=AF.Exp)
            nc.vector.tensor_copy(out=blk[:, NCH:NCH + NH], in_=ex[:])
            den3 = blk[:, NCH:NCH + NH].rearrange(
                "p (h o) -> p h o", o=1).broadcast_to([128, NH, CW])
            b3 = blk[:, 0:NCH].rearrange("p (h c) -> p h c", c=CW)
            nc.vector.tensor_tensor(out=b3, in0=b3, in1=den3, op=OP.mult)
            nc.tensor.matmul(out=acc[:], lhsT=ident_bf[:],
                             rhs=blk[:, 0:NACC], start=True,
                             stop=(remaining[w] == 0))

        def close_window(w):
            acc = acc_tiles.pop(w)
            blk_tiles.pop(w)
            edstb_tiles.pop(w)
            dn = min(WIN, R - w * WIN)
            recip = wp.tile([128, NH], F32, tag="recip")
            nc.vector.reciprocal(out=recip[:], in_=acc[:, NCH:NCH + NH])
            rb = recip[:].rearrange("p (h o) -> p h o", o=1) \
                         .broadcast_to([128, NH, CW])
            if layer == 1:
                hw = h2sb[:, w * NCH:(w + 1) * NCH]
                hw3 = hw.rearrange("p (h c) -> p h c", c=CW)
                ac3 = acc[:, 0:NCH].rearrange("p (h c) -> p h c", c=CW)
                nc.vector.tensor_tensor(out=hw3, in0=ac3, in1=rb, op=OP.mult)
                if consts.get("b1rep") is not None:
                    nc.vector.tensor_tensor(out=hw, in0=hw,
                                            in1=consts["b1rep"][:], op=OP.add)
                # ELU: (max(x,0)-1) + exp(-relu(-x))
                t1 = wp.tile([128, NCH], F32, tag="elu1")
                nc.scalar.activation(out=t1[:], in_=hw, func=AF.Relu,
                                     scale=-1.0)
                nc.scalar.activation(out=t1[:], in_=t1[:], func=AF.Exp,
                                     scale=-1.0)
                nc.vector.tensor_scalar(out=hw, in0=hw, scalar1=0.0,
                                        scalar2=-1.0, op0=OP.max, op1=OP.add)
                nc.vector.tensor_tensor(out=hw, in0=hw, in1=t1[:], op=OP.add)
            else:
                ot = wp.tile([128, cfg.OUT_DIM], F32, tag="ot")
                nc.vector.tensor_scalar_mul(out=ot[:],
                                            in0=acc[:, 0:cfg.OUT_DIM],
                                            scalar1=recip[:, 0:1])
                nc.vector.tensor_tensor(out=ot[:], in0=ot[:], in1=b2rep[:],
                                        op=OP.add)
                nc.scalar.dma_start(out=emb[w * WIN:w * WIN + dn, :],
                                    in_=ot[:dn, :])

        for r in range(rmax):
            idxt = metap.tile([128, NSUB * SEG], I16, tag="idx")
            nc.sync.dma_start(out=idxt[:], in_=gidx_dr[r, :, :])
            mtt = mtpool.tile([128, KF * 128], BF16, tag="mt")
            nc.sync.dma_start(out=mtt[:], in_=gmt_dr[r, :, :])
            gt = ringp.tile([128, KF * CE], BF16, tag="gt")
            for qq in range(NSUB):
                nc.gpsimd.dma_gather(
                    gt[:, qq * K * CE:(qq + 1) * K * CE].rearrange(
                        "p (k c) -> p k c", c=CE),
                    subtabs[qq], idxt[:, qq * SEG:qq * SEG + NIW],
                    NI, NI, CE, queue_num=qq,
                )
            subw = subw_rounds[r]
            for w in sorted(set(subw)):
                if w not in acc_tiles:
                    open_window(w)

            g3 = gt[:].rearrange("p (k c) -> p k c", c=CE)
            mall = mwp.tile([128, KF * 128], BF16, tag="mall")
            for qq in range(NSUB):
                mtp = mtpp.tile([128, K * 128], BF16, tag="mtp",
                                space="PSUM")
                for t in range(K):
                    nc.tensor.transpose(
                        out=mtp[:, t * 128:(t + 1) * 128],
                        in_=mtt[:, (qq * K + t) * 128:(qq * K + t + 1) * 128],
                        identity=ident_bf[:])
                nc.scalar.activation(
                    out=mall[:, qq * K * 128:(qq + 1) * K * 128],
                    in_=mtp[:], func=AF.Copy)
            edp = edpp.tile([128, KF * NH], F32, tag="edp", space="PSUM")
            for t in range(KF):
                nc.tensor.matmul(
                    out=edp[:, t * NH:(t + 1) * NH],
                    lhsT=mtt[:, t * 128:(t + 1) * 128],
                    rhs=edstb_tiles[subw[t]][:],
                    start=True, stop=True)
            lg = wp.tile([128, KF * NH], F32, tag="lg")
            nc.vector.tensor_tensor(
                out=lg[:].rearrange("p (k h) -> p k h", h=NH),
                in0=g3[:, :, NCH:NCH + 2 * NH].bitcast(F32),
                in1=edp[:].rearrange("p (k h) -> p k h", h=NH),
                op=OP.add)
            lr = wp.tile([128, KF * NH], F32, tag="lr")
            nc.scalar.activation(out=lr[:], in_=lg[:], func=AF.Prelu,
                                 alpha=cfg.SLOPE)
            ex = wp.tile([128, KF * NH], F32, tag="ex")
            nc.scalar.activation(out=ex[:], in_=lr[:], func=AF.Exp)
            nc.vector.tensor_copy(
                out=g3[:, :, NCH:NCH + NH],
                in_=ex[:].rearrange("p (k h) -> p k h", h=NH))
            den4 = g3[:, :, NCH:NCH + NH].rearrange(
                "p k (h o) -> p k h o", o=1).broadcast_to([128, KF, NH, CW])
            g4 = g3[:, :, 0:NCH].rearrange("p k (h c) -> p k h c", c=CW)
            nc.vector.tensor_tensor(out=g4, in0=g4, in1=den4, op=OP.mult)
            for t in range(KF):
                w = subw[t]
                nc.tensor.matmul(out=acc_tiles[w][:],
                                 lhsT=mall[:, t * 128:(t + 1) * 128],
                                 rhs=gt[:, t * CE:t * CE + NACC],
                                 start=False,
                                 stop=(remaining[w] == 1))
                remaining[w] -= 1
                if remaining[w] == 0:
                    close_window(w)


def _build_bass(meta, cfg, b1_nonzero, phases=4):
    nc = bacc.Bacc("TRN2", target_bir_lowering=False, debug=False,
                   num_devices=cfg.NCORES, num_swdge_queues=4)
    N, R, NWIN, NSUB, C1 = cfg.N, cfg.R, cfg.NWIN, cfg.NSUB, cfg.C1
    rmax = meta["rmax"]
    NIW = cfg.NI // 16
    SEG = NIW + cfg.KCALL

    x = nc.dram_tensor("x", [N, cfg.IN_DIM], F32, kind="ExternalInput")
    w1x = nc.dram_tensor("w1x", [cfg.IN_DIM, C1 + 8], F32,
                         kind="ExternalInput")
    w2x = nc.dram_tensor("w2x", [128, 2 * (cfg.OUT_DIM + 2)], BF16,
                         kind="ExternalInput")
    identb_in = nc.dram_tensor("identb", [128, 128], BF16,
                               kind="ExternalInput")
    identf_in = nc.dram_tensor("identf", [128, 128], F32,
                               kind="ExternalInput")
    b2rep_in = nc.dram_tensor("b2rep", [128, cfg.OUT_DIM], F32,
                              kind="ExternalInput")
    if b1_nonzero:
        b1rep_in = nc.dram_tensor("b1rep", [128, C1], F32,
                                  kind="ExternalInput")
    gidx_dr = nc.dram_tensor("gidxf", [rmax, 128, NSUB * SEG], I16,
                             kind="ExternalInput")
    gmt_dr = nc.dram_tensor("gmtf", [rmax, 128, NSUB * cfg.KCALL * 128],
                            BF16, kind="ExternalInput")
    emb = nc.dram_tensor("emb", [R, cfg.OUT_DIM], F32, kind="ExternalOutput")

    with tile.TileContext(nc) as tc:
        with tc.tile_pool(name="const", bufs=1) as cp, \
             tc.tile_pool(name="dram", bufs=1, space="DRAM") as dp, \
             tc.tile_pool(name="h2p", bufs=1) as h2p:
            consts = {}
            ident_bf = cp.tile([128, 128], BF16)
            nc.sync.dma_start(out=ident_bf[:], in_=identb_in[:])
            ident_f = cp.tile([128, 128], F32)
            nc.sync.dma_start(out=ident_f[:], in_=identf_in[:])
            w1x_sb = cp.tile([128, C1 + 8], F32)
            nc.sync.dma_start(out=w1x_sb[:], in_=w1x[:])
            w2x_sb = cp.tile([128, 2 * (cfg.OUT_DIM + 2)], BF16)
            nc.sync.dma_start(out=w2x_sb[:], in_=w2x[:])
            b2rep = cp.tile([128, cfg.OUT_DIM], F32)
            nc.sync.dma_start(out=b2rep[:], in_=b2rep_in[:])
            consts["ident_bf"] = ident_bf
            if b1_nonzero:
                b1rep = cp.tile([128, C1], F32)
                nc.sync.dma_start(out=b1rep[:], in_=b1rep_in[:])
                consts["b1rep"] = b1rep
            else:
                consts["b1rep"] = None

            tab1 = dp.tile([N, cfg.CE1], BF16)
            tab2 = dp.tile([N, cfg.CE2], BF16)
            ag_in = dp.tile([R, cfg.CE2], BF16)
            own1 = dp.tile([R, cfg.U1], BF16)
            own2 = dp.tile([R, cfg.U2], BF16)
            h2sb = h2p.tile([128, NWIN * C1], BF16)

            # ---- phase 1: TAB1[n] = [bf16(x@W1), f32 bits of (esrc,edst)] ----
            with tc.tile_pool(name="p1", bufs=4) as p1, \
                 tc.tile_pool(name="p1ps", bufs=2, space="PSUM") as p1ps, \
                 tc.tile_pool(name="p1xp", bufs=3, space="PSUM") as p1xp:
                NMAC = N // 256
                for t2 in range(NMAC):
                    r0 = t2 * 256
                    xt = p1.tile([128, 256], F32, tag="xt")
                    nc.sync.dma_start(
                        out=xt[:].rearrange("p (a c) -> p a c", a=2),
                        in_=x[r0:r0 + 256, :].rearrange("(a p) c -> p a c",
                                                        p=128))
                    hp = p1ps.tile([128, 1024], F32, tag="hp", space="PSUM")
                    for a in range(2):
                        xp = p1xp.tile([128, 128], F32, tag="xp", space="PSUM")
                        nc.tensor.transpose(out=xp[:],
                                            in_=xt[:, a * 128:(a + 1) * 128],
                                            identity=ident_f[:])
                        xT = p1.tile([128, 128], F32, tag="xT")
                        nc.scalar.activation(out=xT[:], in_=xp[:],
                                             func=AF.Copy)
                        nc.tensor.matmul(out=hp[:, a * 512:a * 512 + C1 + 8],
                                         lhsT=xT[:], rhs=w1x_sb[:],
                                         start=True, stop=True)
                    tb = p1.tile([128, 2 * cfg.U1], BF16, tag="tb")
                    tb3 = tb[:].rearrange("p (a u) -> p a u", a=2)
                    hp3 = hp[:].rearrange("p (a u) -> p a u", a=2)
                    nc.scalar.activation(out=tb3[:, :, 0:C1],
                                         in_=hp3[:, :, 0:C1], func=AF.Copy)
                    nc.vector.tensor_copy(
                        out=tb3[:, :, C1:C1 + 16].bitcast(F32),
                        in_=hp3[:, :, C1:C1 + 8])
                    nc.scalar.dma_start(
                        out=tab1[r0:r0 + 256, 0:cfg.U1].rearrange(
                            "(a p) u -> p a u", p=128),
                        in_=tb3)
                for t in range(NMAC * 2, cfg.NT1):
                    dn = min(128, N - t * 128)
                    xt = p1.tile([128, cfg.IN_DIM], F32, tag="xts")
                    if dn < 128:
                        nc.gpsimd.memset(xt[:], 0.0)
                    nc.sync.dma_start(out=xt[:dn, :],
                                      in_=x[t * 128:t * 128 + dn, :])
                    xp = p1xp.tile([128, 128], F32, tag="xp", space="PSUM")
                    nc.tensor.transpose(out=xp[:], in_=xt[:],
                                        identity=ident_f[:])
                    xT = p1.tile([128, 128], F32, tag="xT")
                    nc.scalar.activation(out=xT[:], in_=xp[:], func=AF.Copy)
                    hp = p1ps.tile([128, 1024], F32, tag="hp", space="PSUM")
                    nc.tensor.matmul(out=hp[:, 0:C1 + 8], lhsT=xT[:],
                                     rhs=w1x_sb[:], start=True, stop=True)
                    tb = p1.tile([128, cfg.U1], BF16, tag="tbs")
                    nc.vector.tensor_copy(out=tb[:, 0:C1], in_=hp[:, 0:C1])
                    nc.vector.tensor_copy(out=tb[:, C1:C1 + 16].bitcast(F32),
                                          in_=hp[:, C1:C1 + 8])
                    nc.sync.dma_start(out=tab1[t * 128:t * 128 + dn, 0:cfg.U1],
                                      in_=tb[:dn, :])

            rid = nc.sync.partition_id()
            tab1_3d = tab1[:, 0:cfg.U1].rearrange("(c r) u -> c r u",
                                                  c=cfg.NCORES)
            nc.sync.dma_start(out=own1[:, :].rearrange("(o r) u -> o r u", o=1),
                              in_=tab1_3d[bass.ds(rid, 1), :, :])

            if phases == 1:
                dmy = cp.tile([128, cfg.OUT_DIM], BF16)
                nc.sync.dma_start(out=dmy[:], in_=own1[0:128, 0:cfg.OUT_DIM])
                nc.gpsimd.dma_start(out=emb[0:128, :], in_=dmy[:])
            if phases >= 2:
                # ---- phase 2: layer-1 edge pass -> h2sb ----
                sub1 = [tab1[qq * cfg.SUBN:(qq + 1) * cfg.SUBN, :]
                        for qq in range(NSUB)]
                _edge_pass(nc, cfg, tc, consts, meta, 1, sub1, own1,
                           gidx_dr, gmt_dr, h2sb, emb, b2rep)

            if phases == 2:
                dmy = cp.tile([128, cfg.OUT_DIM], F32)
                nc.vector.tensor_copy(out=dmy[:], in_=h2sb[:, 0:cfg.OUT_DIM])
                nc.sync.dma_start(out=emb[0:128, :], in_=dmy[:])
            # ---- phase 3: TAB2 shard = [bf16(h2@W2x) | f32 bits] + AllGather
            if phases >= 3:
              with tc.tile_pool(name="p4", bufs=4) as p4, \
                 tc.tile_pool(name="p4ps", bufs=3, space="PSUM") as p4ps:
                for t in range(NWIN):
                    dn = min(128, R - t * 128)
                    h2Ts = []
                    for k in range(2):
                        hp2 = p4ps.tile([128, 128], BF16, tag="tps",
                                        space="PSUM")
                        nc.tensor.transpose(
                            out=hp2[:],
                            in_=h2sb[:, t * C1 + k * 128:t * C1 + (k + 1) * 128],
                            identity=ident_bf[:])
                        h2T = p4.tile([128, 128], BF16, tag=f"h2T{k}")
                        nc.scalar.activation(out=h2T[:], in_=hp2[:],
                                             func=AF.Copy)
                        h2Ts.append(h2T)
                    W = cfg.OUT_DIM + 2
                    hh = p4ps.tile([128, W], F32, tag="hh", space="PSUM")
                    nc.tensor.matmul(out=hh[:], lhsT=h2Ts[0][:],
                                     rhs=w2x_sb[:, 0:W], start=True,
                                     stop=False)
                    nc.tensor.matmul(out=hh[:], lhsT=h2Ts[1][:],
                                     rhs=w2x_sb[:, W:2 * W], start=False,
                                     stop=True)
                    t2b = p4.tile([128, cfg.U2], BF16, tag="t2b")
                    nc.vector.tensor_copy(out=t2b[:, 0:cfg.OUT_DIM],
                                          in_=hh[:, 0:cfg.OUT_DIM])
                    nc.vector.tensor_copy(
                        out=t2b[:, cfg.OUT_DIM:cfg.OUT_DIM + 4].bitcast(F32),
                        in_=hh[:, cfg.OUT_DIM:cfg.OUT_DIM + 2])
                    nc.scalar.dma_start(
                        out=ag_in[t * 128:t * 128 + dn, 0:cfg.U2],
                        in_=t2b[:dn, :])
            if phases >= 3:
                nc.gpsimd.collective_compute(
                    "AllGather", OP.bypass,
                    replica_groups=[list(range(cfg.NCORES))],
                    ins=[ag_in[:, :]], outs=[tab2[:, :]],
                )
                tab2_3d = tab2[:, 0:cfg.U2].rearrange("(c r) u -> c r u",
                                                      c=cfg.NCORES)
                nc.sync.dma_start(
                    out=own2[:, :].rearrange("(o r) u -> o r u", o=1),
                    in_=tab2_3d[bass.ds(rid, 1), :, :])
            if phases == 3:
                dmy = cp.tile([128, cfg.OUT_DIM], BF16)
                nc.sync.dma_start(out=dmy[:], in_=own2[0:128, 0:cfg.OUT_DIM])
                nc.gpsimd.dma_start(out=emb[0:128, :], in_=dmy[:])
            if phases >= 4:
                # ---- phase 4: layer-2 edge pass -> emb ----
                sub2 = [tab2[qq * cfg.SUBN:(qq + 1) * cfg.SUBN, :]
                        for qq in range(NSUB)]
                _edge_pass(nc, cfg, tc, consts, meta, 2, sub2, own2,
                           gidx_dr, gmt_dr, h2sb, emb, b2rep)

    nc.compile()
    return nc


def _host_inputs(meta, cfg, x, W1, a_src1, a_dst1, b1, W2, a_src2, a_dst2, b2):
    C1 = cfg.C1
    w1e = np.zeros((cfg.IN_DIM, 8), np.float32)
    for j in range(cfg.HEADS):
        blkw = W1[:, j * cfg.HID:(j + 1) * cfg.HID]
        w1e[:, j] = blkw @ a_src1[j]
        w1e[:, 4 + j] = blkw @ a_dst1[j]
    w1x = np.concatenate([W1, w1e], axis=1).astype(np.float32)
    W = cfg.OUT_DIM + 2
    w2full = np.concatenate(
        [W2, (W2 @ a_src2[0])[:, None], (W2 @ a_dst2[0])[:, None]],
        axis=1).astype(np.float32)                      # [256, 34]
    w2x = np.concatenate([w2full[:128], w2full[128:]], axis=1)  # [128, 68]
    w2x = w2x.astype(ml_dtypes.bfloat16)

    identb = np.eye(128, dtype=np.float32).astype(ml_dtypes.bfloat16)
    identf = np.eye(128, dtype=np.float32)
    b2rep = np.tile(np.asarray(b2, np.float32)[None, :], (128, 1))
    base = {"x": np.asarray(x, np.float32), "w1x": w1x, "w2x": w2x,
            "identb": identb, "identf": identf, "b2rep": b2rep}
    if np.any(b1):
        base["b1rep"] = np.tile(np.asarray(b1, np.float32)[None, :], (128, 1))
    in_maps = []
    for c in range(cfg.NCORES):
        m = dict(base)
        m["gidxf"] = meta["per_core"][c]["gidxf"]
        m["gmtf"] = meta["per_core"][c]["gmtf"]
        in_maps.append(m)
    return in_maps


_CACHE = {}


def _get_compiled(edge_index, cfg, b1_nonzero):
    key = (hashlib.sha1(np.ascontiguousarray(edge_index).tobytes())
           .hexdigest(), b1_nonzero, cfg.N)
    if key not in _CACHE:
        meta = _build_meta(edge_index, cfg)
        nc = _build_bass(meta, cfg, b1_nonzero)
        _CACHE[key] = (meta, nc)
    return _CACHE[key]


def kernel(x, edge_index, W1, a_src1, a_dst1, b1, W2, a_src2, a_dst2, b2):
    cfg = Cfg()
    x = np.asarray(x)
    edge_index = np.asarray(edge_index)
    meta, nc = _get_compiled(edge_index, cfg, bool(np.any(np.asarray(b1))))
    in_maps = _host_inputs(meta, cfg, x, np.asarray(W1), np.asarray(a_src1),
                           np.asarray(a_dst1), np.asarray(b1), np.asarray(W2),
                           np.asarray(a_src2), np.asarray(a_dst2),
                           np.asarray(b2))
    res = run_bass_kernel_spmd(nc, in_maps, list(range(cfg.NCORES)))
    shards = [res.results[c]["emb"] for c in range(cfg.NCORES)]
    return np.concatenate(shards, axis=0)


# revision 6
# speedup vs baseline: 45.1214x; 2.2404x over previous
"""Trainium2 Bass kernel for a 2-layer GAT (nn_DepthAwareGAT).

Self-contained: hardcodes problem shapes; accepts FULL inputs, returns FULL
output. Strategy:
  - dst-range sharding across 8 cores (12500 nodes each; edge counts balance
    to ~0.5% since dst is uniform).
  - Node tables TAB1/TAB2 hold per-node rows [h | raw-f32-bits of
    (e_src, e_dst)] in bf16; per-edge h[src] rows fetched with int16
    dma_gather (4 SWDGE queues, 4 src-range subtables).
  - Segment softmax+aggregate in ONE pass per layer via mask matmuls:
    out[d] = (sum_e exp(e_e) h[src_e]) / (sum_e exp(e_e)); no segment max
    needed (logits are O(1), exp cannot overflow).
  - e_dst per edge expanded on-chip: MT = host-shipped onehot(dst_local)^T,
    e_dst_edge = MT @ e_dst_window; M = PE-transpose of MT feeds the
    accumulate matmuls.
  - Fused rounds: the 4 subtable streams share one index DMA, one mask DMA,
    one gather tile and single wide vector/scalar ops per round (4096 edges)
    to amortize per-op overheads.
  - h2 = elu(out1) kept on-chip (bf16); TAB2 shards AllGathered between
    layers; windows read TAB1/TAB2 directly at a dynamic core-id offset.
"""

import hashlib

import numpy as np
import ml_dtypes

import concourse.bacc as bacc
import concourse.bass as bass
import concourse.tile as tile
import concourse.mybir as mybir
from concourse.bass_utils import run_bass_kernel_spmd

F32 = mybir.dt.float32
BF16 = mybir.dt.bfloat16
I16 = mybir.dt.int16
AF = mybir.ActivationFunctionType
OP = mybir.AluOpType


class Cfg:
    def __init__(self, N=100000, IN_DIM=128, HID=64, HEADS=4, OUT_DIM=32):
        self.N = N
        self.IN_DIM = IN_DIM
        self.HID = HID
        self.HEADS = HEADS
        self.OUT_DIM = OUT_DIM
        self.SLOPE = 0.2
        self.NCORES = 8
        self.WIN = 128
        self.NSUB = 4
        self.NI = 1024
        self.PADDL = 200.0
        self.RING = 3
        self.C1 = HID * HEADS                       # 256
        self.CE1 = 384                              # bf16 row: 768B (%256 ok)
        self.U1 = self.C1 + 16                      # used cols: h + 8 f32 bits
        self.CE2 = 128                              # 256B
        self.U2 = OUT_DIM + 4                       # h2 + 2 f32 bits
        self.R = N // self.NCORES
        self.NWIN = (self.R + self.WIN - 1) // self.WIN
        self.SUBN = N // self.NSUB
        self.KCALL = self.NI // 128
        self.NT1 = (N + 127) // 128


def _wrap_calls(flat):
    """[ncalls, NI] int -> [ncalls, 128, NI//16] int16 wrapped layout."""
    nc_, ni = flat.shape
    a = flat.reshape(nc_, ni // 16, 16).transpose(0, 2, 1)
    return np.tile(a, (1, 8, 1)).astype(np.int16)


def _build_meta(edge_index, cfg):
    """Host-side prep: per-core fused round-major gather streams + masks."""
    src = np.asarray(edge_index[0], np.int64)
    dst = np.asarray(edge_index[1], np.int64)
    R, WIN, NWIN, NSUB = cfg.R, cfg.WIN, cfg.NWIN, cfg.NSUB
    NI, KCALL = cfg.NI, cfg.KCALL
    NIW = NI // 16

    core = dst // R
    w_all = (dst % R) // WIN
    q_all = src // cfg.SUBN
    counts = np.zeros((cfg.NCORES, NWIN, NSUB), np.int64)
    np.add.at(counts, (core, w_all, q_all), 1)
    kwq = (-(-counts // 128)).max(axis=0)         # [NWIN, NSUB]
    nsub_q = kwq.sum(axis=0)
    ncalls_q = -(-nsub_q // KCALL)
    ncalls_q = np.maximum(ncalls_q, 1)
    rmax = int(ncalls_q.max())
    nsub_pad_q = np.full(NSUB, rmax * KCALL, np.int64)
    off = np.zeros((NWIN, NSUB), np.int64)
    off[1:] = np.cumsum(kwq, axis=0)[:-1]

    per_core = []
    for c in range(cfg.NCORES):
        sel = core == c
        s = src[sel]
        d = dst[sel] - c * R
        w = d // WIN
        dl = (d % WIN).astype(np.float32)
        q = s // cfg.SUBN
        sl = (s % cfg.SUBN).astype(np.int64)
        o = np.lexsort((sl, q, w))
        w, dl, q, sl = w[o], dl[o], q[o], sl[o]
        gidx_q, gmt_q = [], []
        for qq in range(NSUB):
            m = q == qq
            wq, dlq, slq = w[m], dl[m], sl[m]
            nslots = int(nsub_pad_q[qq]) * 128
            slots_i = np.zeros(nslots, np.int64)
            slots_d = np.full(nslots, cfg.PADDL, np.float32)
            grp_start = np.searchsorted(wq, np.arange(NWIN))
            pos = off[wq, qq] * 128 + (np.arange(len(wq)) - grp_start[wq])
            slots_i[pos] = slq
            slots_d[pos] = dlq
            wrapped = _wrap_calls(slots_i.reshape(rmax, NI))
            dlb = (slots_d.reshape(rmax, KCALL, 128).transpose(0, 2, 1)
                   .astype(ml_dtypes.bfloat16))
            gidx_q.append(np.concatenate([wrapped, dlb.view(np.int16)],
                                         axis=2))
            # transposed one-hot masks MT[d, e] per subtile, bf16
            dlv = slots_d.reshape(rmax, KCALL, 128)      # [r, t, e]
            mt = (np.arange(128, dtype=np.float32)[None, None, :, None]
                  == dlv[:, :, None, :])                 # [r, t, d, e]
            mt = (mt.transpose(0, 2, 1, 3)               # [r, d, t, e]
                  .reshape(rmax, 128, KCALL * 128)
                  .astype(ml_dtypes.bfloat16))
            gmt_q.append(mt)
        gidxf = np.concatenate(gidx_q, axis=2)           # [r, 128, 4*(NIW+K)]
        gmtf = np.concatenate(gmt_q, axis=2)             # [r, 128, 4*K*128]
        per_core.append({"gidxf": np.ascontiguousarray(gidxf),
                         "gmtf": np.ascontiguousarray(gmtf)})

    # window of each (stream, subtile); tail dummies -> last window
    win_of_sub = []
    for qq in range(NSUB):
        lst = []
        for w in range(NWIN):
            lst.extend([w] * int(kwq[w, qq]))
        lst.extend([NWIN - 1] * int(nsub_pad_q[qq] - len(lst)))
        win_of_sub.append(lst)

    # per-round window list across streams (gt-column order), plus max
    # concurrently-open windows for PSUM sizing
    subw_rounds = []
    for r in range(rmax):
        subw_rounds.append([win_of_sub[qq][r * KCALL + t]
                            for qq in range(NSUB) for t in range(KCALL)])
    remaining = np.zeros(NWIN, np.int64)
    for row in subw_rounds:
        for w in row:
            remaining[w] += 1
    open_set, max_open = set(), 0
    rem = remaining.copy()
    for row in subw_rounds:
        for w in set(row):
            open_set.add(w)
        max_open = max(max_open, len(open_set))
        for w in row:
            rem[w] -= 1
            if rem[w] == 0:
                open_set.discard(w)
    return {"per_core": per_core, "subw_rounds": subw_rounds,
            "rmax": rmax, "max_open": max_open}


def _edge_pass(nc, cfg, tc, consts, meta, layer, subtabs, own, gidx_dr,
               gmt_dr, h2sb, emb, b2rep):
    """One edge-parallel layer pass; fused rounds over the 4 streams."""
    R, WIN, NWIN, NSUB = cfg.R, cfg.WIN, cfg.NWIN, cfg.NSUB
    NI, K = cfg.NI, cfg.KCALL
    KF = NSUB * K                                   # subtiles per round
    CE = cfg.CE1 if layer == 1 else cfg.CE2
    U = cfg.U1 if layer == 1 else cfg.U2
    NCH = cfg.C1 if layer == 1 else cfg.OUT_DIM
    NH = cfg.HEADS if layer == 1 else 1
    CW = NCH // NH
    NACC = NCH + NH
    ident_bf = consts["ident_bf"]
    subw_rounds = meta["subw_rounds"]
    rmax = meta["rmax"]
    NIW = NI // 16
    SEG = NIW + K                                   # idxt cols per stream

    remaining = [0] * NWIN
    for row in subw_rounds:
        for w in row:
            remaining[w] += 1

    accbufs = max(5, meta["max_open"] + 1)
    with tc.tile_pool(name=f"ring{layer}", bufs=cfg.RING) as ringp, \
         tc.tile_pool(name=f"meta{layer}", bufs=3) as metap, \
         tc.tile_pool(name=f"mt{layer}", bufs=2) as mtpool, \
         tc.tile_pool(name=f"work{layer}", bufs=4) as wp, \
         tc.tile_pool(name=f"mwork{layer}", bufs=2) as mwp, \
         tc.tile_pool(name=f"blk{layer}", bufs=4) as blkp, \
         tc.tile_pool(name=f"accps{layer}", bufs=accbufs,
                      space="PSUM") as accp, \
         tc.tile_pool(name=f"mtps{layer}", bufs=2, space="PSUM") as mtpp, \
         tc.tile_pool(name=f"edps{layer}", bufs=1, space="PSUM") as edpp:

        blk_tiles, acc_tiles, edstb_tiles = {}, {}, {}

        def open_window(w):
            dn = min(WIN, R - w * WIN)
            blk = blkp.tile([128, U], BF16, tag="blk")
            if dn < 128:
                nc.gpsimd.memset(blk[:], 0.0)
            nc.sync.dma_start(out=blk[:dn, :],
                              in_=own[w * WIN:w * WIN + dn, 0:U])
            acc = accp.tile([128, NACC], F32, tag="acc", space="PSUM")
            blk_tiles[w] = blk
            acc_tiles[w] = acc
            esrc_self = blk[:, NCH:NCH + 2 * NH].bitcast(F32)
            edst_w = blk[:, NCH + 2 * NH:NCH + 4 * NH].bitcast(F32)
            edstb = blkp.tile([128, NH], BF16, tag="edstb")
            nc.vector.tensor_copy(out=edstb[:], in_=edst_w)
            edstb_tiles[w] = edstb
            lg = wp.tile([128, NH], F32, tag="lg0")
            nc.vector.tensor_tensor(out=lg[:], in0=esrc_self, in1=edst_w,
                                    op=OP.add)
            lr = wp.tile([128, NH], F32, tag="lr0")
            nc.scalar.activation(out=lr[:], in_=lg[:], func=AF.Prelu,
                                 alpha=cfg.SLOPE)
            ex = wp.tile([128, NH], F32, tag="ex0")
            nc.scalar.activation(out=ex[:], in_=lr[:], func=AF.Exp)
            nc.vector.tensor_copy(out=blk[:, NCH:NCH + NH], in_=ex[:])
            den3 = blk[:, NCH:NCH + NH].rearrange(
                "p (h o) -> p h o", o=1).broadcast_to([128, NH, CW])
            b3 = blk[:, 0:NCH].rearrange("p (h c) -> p h c", c=CW)
            nc.vector.tensor_tensor(out=b3, in0=b3, in1=den3, op=OP.mult)
            nc.tensor.matmul(out=acc[:], lhsT=ident_bf[:],
                             rhs=blk[:, 0:NACC], start=True,
                             stop=(remaining[w] == 0))

        def close_window(w):
            acc = acc_tiles.pop(w)
            blk_tiles.pop(w)
            edstb_tiles.pop(w)
            dn = min(WIN, R - w * WIN)
            recip = wp.tile([128, NH], F32, tag="recip")
            nc.vector.reciprocal(out=recip[:], in_=acc[:, NCH:NCH + NH])
            rb = recip[:].rearrange("p (h o) -> p h o", o=1) \
                         .broadcast_to([128, NH, CW])
            if layer == 1:
                hw = h2sb[:, w * NCH:(w + 1) * NCH]
                hw3 = hw.rearrange("p (h c) -> p h c", c=CW)
                ac3 = acc[:, 0:NCH].rearrange("p (h c) -> p h c", c=CW)
                nc.vector.tensor_tensor(out=hw3, in0=ac3, in1=rb, op=OP.mult)
                if consts.get("b1rep") is not None:
                    nc.vector.tensor_tensor(out=hw, in0=hw,
                                            in1=consts["b1rep"][:], op=OP.add)
                # ELU: (max(x,0)-1) + exp(-relu(-x))
                t1 = wp.tile([128, NCH], F32, tag="elu1")
                nc.scalar.activation(out=t1[:], in_=hw, func=AF.Relu,
                                     scale=-1.0)
                nc.scalar.activation(out=t1[:], in_=t1[:], func=AF.Exp,
                                     scale=-1.0)
                nc.vector.tensor_scalar(out=hw, in0=hw, scalar1=0.0,
                                        scalar2=-1.0, op0=OP.max, op1=OP.add)
                nc.vector.tensor_tensor(out=hw, in0=hw, in1=t1[:], op=OP.add)
            else:
                ot = wp.tile([128, cfg.OUT_DIM], F32, tag="ot")
                nc.vector.tensor_scalar_mul(out=ot[:],
                                            in0=acc[:, 0:cfg.OUT_DIM],
                                            scalar1=recip[:, 0:1])
                nc.vector.tensor_tensor(out=ot[:], in0=ot[:], in1=b2rep[:],
                                        op=OP.add)
                nc.scalar.dma_start(out=emb[w * WIN:w * WIN + dn, :],
                                    in_=ot[:dn, :])

        for r in range(rmax):
            idxt = metap.tile([128, NSUB * SEG], I16, tag="idx")
            nc.sync.dma_start(out=idxt[:], in_=gidx_dr[r, :, :])
            mtt = mtpool.tile([128, KF * 128], BF16, tag="mt")
            nc.sync.dma_start(out=mtt[:], in_=gmt_dr[r, :, :])
            gt = ringp.tile([128, KF * CE], BF16, tag="gt")
            for qq in range(NSUB):
                nc.gpsimd.dma_gather(
                    gt[:, qq * K * CE:(qq + 1) * K * CE].rearrange(
                        "p (k c) -> p k c", c=CE),
                    subtabs[qq], idxt[:, qq * SEG:qq * SEG + NIW],
                    NI, NI, CE, queue_num=qq,
                )
            subw = subw_rounds[r]
            for w in sorted(set(subw)):
                if w not in acc_tiles:
                    open_window(w)

            g3 = gt[:].rearrange("p (k c) -> p k c", c=CE)
            mall = mwp.tile([128, KF * 128], BF16, tag="mall")
            for qq in range(NSUB):
                mtp = mtpp.tile([128, K * 128], BF16, tag="mtp",
                                space="PSUM")
                for t in range(K):
                    nc.tensor.transpose(
                        out=mtp[:, t * 128:(t + 1) * 128],
                        in_=mtt[:, (qq * K + t) * 128:(qq * K + t + 1) * 128],
                        identity=ident_bf[:])
                nc.scalar.activation(
                    out=mall[:, qq * K * 128:(qq + 1) * K * 128],
                    in_=mtp[:], func=AF.Copy)
            edp = edpp.tile([128, KF * NH], F32, tag="edp", space="PSUM")
            for t in range(KF):
                nc.tensor.matmul(
                    out=edp[:, t * NH:(t + 1) * NH],
                    lhsT=mtt[:, t * 128:(t + 1) * 128],
                    rhs=edstb_tiles[subw[t]][:],
                    start=True, stop=True)
            lg = wp.tile([128, KF * NH], F32, tag="lg")
            nc.vector.tensor_tensor(
                out=lg[:].rearrange("p (k h) -> p k h", h=NH),
                in0=g3[:, :, NCH:NCH + 2 * NH].bitcast(F32),
                in1=edp[:].rearrange("p (k h) -> p k h", h=NH),
                op=OP.add)
            lr = wp.tile([128, KF * NH], F32, tag="lr")
            nc.scalar.activation(out=lr[:], in_=lg[:], func=AF.Prelu,
                                 alpha=cfg.SLOPE)
            ex = wp.tile([128, KF * NH], F32, tag="ex")
            nc.scalar.activation(out=ex[:], in_=lr[:], func=AF.Exp)
            nc.vector.tensor_copy(
                out=g3[:, :, NCH:NCH + NH],
                in_=ex[:].rearrange("p (k h) -> p k h", h=NH))
            den4 = g3[:, :, NCH:NCH + NH].rearrange(
                "p k (h o) -> p k h o", o=1).broadcast_to([128, KF, NH, CW])
            g4 = g3[:, :, 0:NCH].rearrange("p k (h c) -> p k h c", c=CW)
            nc.vector.tensor_tensor(out=g4, in0=g4, in1=den4, op=OP.mult)
            for t in range(KF):
                w = subw[t]
                nc.tensor.matmul(out=acc_tiles[w][:],
                                 lhsT=mall[:, t * 128:(t + 1) * 128],
                                 rhs=gt[:, t * CE:t * CE + NACC],
                                 start=False,
                                 stop=(remaining[w] == 1))
                remaining[w] -= 1
                if remaining[w] == 0:
                    close_window(w)


def _build_bass(meta, cfg, b1_nonzero, phases=4):
    nc = bacc.Bacc("TRN2", target_bir_lowering=False, debug=False,
                   num_devices=cfg.NCORES, num_swdge_queues=4)
    N, R, NWIN, NSUB, C1 = cfg.N, cfg.R, cfg.NWIN, cfg.NSUB, cfg.C1
    rmax = meta["rmax"]
    NIW = cfg.NI // 16
    SEG = NIW + cfg.KCALL

    x = nc.dram_tensor("x", [N, cfg.IN_DIM], F32, kind="ExternalInput")
    w1x = nc.dram_tensor("w1x", [cfg.IN_DIM, C1 + 8], BF16,
                         kind="ExternalInput")
    w2x = nc.dram_tensor("w2x", [128, 2 * (cfg.OUT_DIM + 2)], BF16,
                         kind="ExternalInput")
    identb_in = nc.dram_tensor("identb", [128, 128], BF16,
                               kind="ExternalInput")
    identf_in = nc.dram_tensor("identf", [128, 128], F32,
                               kind="ExternalInput")
    b2rep_in = nc.dram_tensor("b2rep", [128, cfg.OUT_DIM], F32,
                              kind="ExternalInput")
    if b1_nonzero:
        b1rep_in = nc.dram_tensor("b1rep", [128, C1], F32,
                                  kind="ExternalInput")
    gidx_dr = nc.dram_tensor("gidxf", [rmax, 128, NSUB * SEG], I16,
                             kind="ExternalInput")
    gmt_dr = nc.dram_tensor("gmtf", [rmax, 128, NSUB * cfg.KCALL * 128],
                            BF16, kind="ExternalInput")
    emb = nc.dram_tensor("emb", [R, cfg.OUT_DIM], F32, kind="ExternalOutput")

    with tile.TileContext(nc) as tc:
        with tc.tile_pool(name="const", bufs=1) as cp, \
             tc.tile_pool(name="dram", bufs=1, space="DRAM") as dp, \
             tc.tile_pool(name="h2p", bufs=1) as h2p:
            consts = {}
            ident_bf = cp.tile([128, 128], BF16)
            nc.sync.dma_start(out=ident_bf[:], in_=identb_in[:])
            ident_f = cp.tile([128, 128], F32)
            nc.sync.dma_start(out=ident_f[:], in_=identf_in[:])
            w1x_sb = cp.tile([128, C1 + 8], BF16)
            nc.sync.dma_start(out=w1x_sb[:], in_=w1x[:])
            w2x_sb = cp.tile([128, 2 * (cfg.OUT_DIM + 2)], BF16)
            nc.sync.dma_start(out=w2x_sb[:], in_=w2x[:])
            b2rep = cp.tile([128, cfg.OUT_DIM], F32)
            nc.sync.dma_start(out=b2rep[:], in_=b2rep_in[:])
            consts["ident_bf"] = ident_bf
            if b1_nonzero:
                b1rep = cp.tile([128, C1], F32)
                nc.sync.dma_start(out=b1rep[:], in_=b1rep_in[:])
                consts["b1rep"] = b1rep
            else:
                consts["b1rep"] = None

            tab1 = dp.tile([N, cfg.CE1], BF16)
            tab2 = dp.tile([N, cfg.CE2], BF16)
            ag_in = dp.tile([R, cfg.CE2], BF16)
            own1 = dp.tile([R, cfg.U1], BF16)
            own2 = dp.tile([R, cfg.U2], BF16)
            h2sb = h2p.tile([128, NWIN * C1], BF16)

            # ---- phase 1: TAB1[n] = [bf16(x@W1), f32 bits of (esrc,edst)] ----
            with tc.tile_pool(name="p1", bufs=4) as p1, \
                 tc.tile_pool(name="p1ps", bufs=2, space="PSUM") as p1ps, \
                 tc.tile_pool(name="p1xp", bufs=3, space="PSUM") as p1xp:
                NMAC = N // 256
                for t2 in range(NMAC):
                    r0 = t2 * 256
                    xt = p1.tile([128, 256], F32, tag="xt")
                    nc.sync.dma_start(
                        out=xt[:].rearrange("p (a c) -> p a c", a=2),
                        in_=x[r0:r0 + 256, :].rearrange("(a p) c -> p a c",
                                                        p=128))
                    hp = p1ps.tile([128, 1024], F32, tag="hp", space="PSUM")
                    for a in range(2):
                        xp = p1xp.tile([128, 128], F32, tag="xp", space="PSUM")
                        nc.tensor.transpose(out=xp[:],
                                            in_=xt[:, a * 128:(a + 1) * 128],
                                            identity=ident_f[:])
                        xT = p1.tile([128, 128], BF16, tag="xT")
                        nc.scalar.activation(out=xT[:], in_=xp[:],
                                             func=AF.Copy)
                        nc.tensor.matmul(out=hp[:, a * 512:a * 512 + C1 + 8],
                                         lhsT=xT[:], rhs=w1x_sb[:],
                                         start=True, stop=True)
                    tb = p1.tile([128, 2 * cfg.U1], BF16, tag="tb")
                    tb3 = tb[:].rearrange("p (a u) -> p a u", a=2)
                    hp3 = hp[:].rearrange("p (a u) -> p a u", a=2)
                    nc.scalar.activation(out=tb3[:, :, 0:C1],
                                         in_=hp3[:, :, 0:C1], func=AF.Copy)
                    nc.vector.tensor_copy(
                        out=tb3[:, :, C1:C1 + 16].bitcast(F32),
                        in_=hp3[:, :, C1:C1 + 8])
                    nc.scalar.dma_start(
                        out=tab1[r0:r0 + 256, 0:cfg.U1].rearrange(
                            "(a p) u -> p a u", p=128),
                        in_=tb3)
                for t in range(NMAC * 2, cfg.NT1):
                    dn = min(128, N - t * 128)
                    xt = p1.tile([128, cfg.IN_DIM], F32, tag="xts")
                    if dn < 128:
                        nc.gpsimd.memset(xt[:], 0.0)
                    nc.sync.dma_start(out=xt[:dn, :],
                                      in_=x[t * 128:t * 128 + dn, :])
                    xp = p1xp.tile([128, 128], F32, tag="xp", space="PSUM")
                    nc.tensor.transpose(out=xp[:], in_=xt[:],
                                        identity=ident_f[:])
                    xT = p1.tile([128, 128], BF16, tag="xT")
                    nc.scalar.activation(out=xT[:], in_=xp[:], func=AF.Copy)
                    hp = p1ps.tile([128, 1024], F32, tag="hp", space="PSUM")
                    nc.tensor.matmul(out=hp[:, 0:C1 + 8], lhsT=xT[:],
                                     rhs=w1x_sb[:], start=True, stop=True)
                    tb = p1.tile([128, cfg.U1], BF16, tag="tbs")
                    nc.vector.tensor_copy(out=tb[:, 0:C1], in_=hp[:, 0:C1])
                    nc.vector.tensor_copy(out=tb[:, C1:C1 + 16].bitcast(F32),
                                          in_=hp[:, C1:C1 + 8])
                    nc.sync.dma_start(out=tab1[t * 128:t * 128 + dn, 0:cfg.U1],
                                      in_=tb[:dn, :])

            rid = nc.sync.partition_id()
            tab1_3d = tab1[:, 0:cfg.U1].rearrange("(c r) u -> c r u",
                                                  c=cfg.NCORES)
            nc.sync.dma_start(out=own1[:, :].rearrange("(o r) u -> o r u", o=1),
                              in_=tab1_3d[bass.ds(rid, 1), :, :])

            if phases == 1:
                dmy = cp.tile([128, cfg.OUT_DIM], BF16)
                nc.sync.dma_start(out=dmy[:], in_=own1[0:128, 0:cfg.OUT_DIM])
                nc.gpsimd.dma_start(out=emb[0:128, :], in_=dmy[:])
            if phases >= 2:
                # ---- phase 2: layer-1 edge pass -> h2sb ----
                sub1 = [tab1[qq * cfg.SUBN:(qq + 1) * cfg.SUBN, :]
                        for qq in range(NSUB)]
                _edge_pass(nc, cfg, tc, consts, meta, 1, sub1, own1,
                           gidx_dr, gmt_dr, h2sb, emb, b2rep)

            if phases == 2:
                dmy = cp.tile([128, cfg.OUT_DIM], F32)
                nc.vector.tensor_copy(out=dmy[:], in_=h2sb[:, 0:cfg.OUT_DIM])
                nc.sync.dma_start(out=emb[0:128, :], in_=dmy[:])
            # ---- phase 3: TAB2 shard = [bf16(h2@W2x) | f32 bits] + AllGather
            if phases >= 3:
              with tc.tile_pool(name="p4", bufs=4) as p4, \
                 tc.tile_pool(name="p4ps", bufs=3, space="PSUM") as p4ps:
                for t in range(NWIN):
                    dn = min(128, R - t * 128)
                    h2Ts = []
                    for k in range(2):
                        hp2 = p4ps.tile([128, 128], BF16, tag="tps",
                                        space="PSUM")
                        nc.tensor.transpose(
                            out=hp2[:],
                            in_=h2sb[:, t * C1 + k * 128:t * C1 + (k + 1) * 128],
                            identity=ident_bf[:])
                        h2T = p4.tile([128, 128], BF16, tag=f"h2T{k}")
                        nc.scalar.activation(out=h2T[:], in_=hp2[:],
                                             func=AF.Copy)
                        h2Ts.append(h2T)
                    W = cfg.OUT_DIM + 2
                    hh = p4ps.tile([128, W], F32, tag="hh", space="PSUM")
                    nc.tensor.matmul(out=hh[:], lhsT=h2Ts[0][:],
                                     rhs=w2x_sb[:, 0:W], start=True,
                                     stop=False)
                    nc.tensor.matmul(out=hh[:], lhsT=h2Ts[1][:],
                                     rhs=w2x_sb[:, W:2 * W], start=False,
                                     stop=True)
                    t2b = p4.tile([128, cfg.U2], BF16, tag="t2b")
                    nc.vector.tensor_copy(out=t2b[:, 0:cfg.OUT_DIM],
                                          in_=hh[:, 0:cfg.OUT_DIM])
                    nc.vector.tensor_copy(
                        out=t2b[:, cfg.OUT_DIM:cfg.OUT_DIM + 4].bitcast(F32),
                        in_=hh[:, cfg.OUT_DIM:cfg.OUT_DIM + 2])
                    nc.scalar.dma_start(
                        out=ag_in[t * 128:t * 128 + dn, 0:cfg.U2],
                        in_=t2b[:dn, :])
            if phases >= 3:
                nc.gpsimd.collective_compute(
                    "AllGather", OP.bypass,
                    replica_groups=[list(range(cfg.NCORES))],
                    ins=[ag_in[:, :]], outs=[tab2[:, :]],
                )
                tab2_3d = tab2[:, 0:cfg.U2].rearrange("(c r) u -> c r u",
                                                      c=cfg.NCORES)
                nc.sync.dma_start(
                    out=own2[:, :].rearrange("(o r) u -> o r u", o=1),
                    in_=tab2_3d[bass.ds(rid, 1), :, :])
            if phases == 3:
                dmy = cp.tile([128, cfg.OUT_DIM], BF16)
                nc.sync.dma_start(out=dmy[:], in_=own2[0:128, 0:cfg.OUT_DIM])
                nc.gpsimd.dma_start(out=emb[0:128, :], in_=dmy[:])
            if phases >= 4:
                # ---- phase 4: layer-2 edge pass -> emb ----
                sub2 = [tab2[qq * cfg.SUBN:(qq + 1) * cfg.SUBN, :]
                        for qq in range(NSUB)]
                _edge_pass(nc, cfg, tc, consts, meta, 2, sub2, own2,
                           gidx_dr, gmt_dr, h2sb, emb, b2rep)

    nc.compile()
    return nc


def _host_inputs(meta, cfg, x, W1, a_src1, a_dst1, b1, W2, a_src2, a_dst2, b2):
    C1 = cfg.C1
    w1e = np.zeros((cfg.IN_DIM, 8), np.float32)
    for j in range(cfg.HEADS):
        blkw = W1[:, j * cfg.HID:(j + 1) * cfg.HID]
        w1e[:, j] = blkw @ a_src1[j]
        w1e[:, 4 + j] = blkw @ a_dst1[j]
    w1x = np.concatenate([W1, w1e], axis=1).astype(ml_dtypes.bfloat16)
    W = cfg.OUT_DIM + 2
    w2full = np.concatenate(
        [W2, (W2 @ a_src2[0])[:, None], (W2 @ a_dst2[0])[:, None]],
        axis=1).astype(np.float32)                      # [256, 34]
    w2x = np.concatenate([w2full[:128], w2full[128:]], axis=1)  # [128, 68]
    w2x = w2x.astype(ml_dtypes.bfloat16)

    identb = np.eye(128, dtype=np.float32).astype(ml_dtypes.bfloat16)
    identf = np.eye(128, dtype=np.float32)
    b2rep = np.tile(np.asarray(b2, np.float32)[None, :], (128, 1))
    base = {"x": np.asarray(x, np.float32), "w1x": w1x, "w2x": w2x,
            "identb": identb, "identf": identf, "b2rep": b2rep}
    if np.any(b1):
        base["b1rep"] = np.tile(np.asarray(b1, np.float32)[None, :], (128, 1))
    in_maps = []
    for c in range(cfg.NCORES):
        m = dict(base)
        m["gidxf"] = meta["per_core"][c]["gidxf"]
        m["gmtf"] = meta["per_core"][c]["gmtf"]
        in_maps.append(m)
    return in_maps


_CACHE = {}


def _get_compiled(edge_index, cfg, b1_nonzero):
    key = (hashlib.sha1(np.ascontiguousarray(edge_index).tobytes())
           .hexdigest(), b1_nonzero, cfg.N)
    if key not in _CACHE:
        meta = _build_meta(edge_index, cfg)
        nc = _build_bass(meta, cfg, b1_nonzero)
        _CACHE[key] = (meta, nc)
    return _CACHE[key]


def kernel(x, edge_index, W1, a_src1, a_dst1, b1, W2, a_src2, a_dst2, b2):
    cfg = Cfg()
    x = np.asarray(x)
    edge_index = np.asarray(edge_index)
    meta, nc = _get_compiled(edge_index, cfg, bool(np.any(np.asarray(b1))))
    in_maps = _host_inputs(meta, cfg, x, np.asarray(W1), np.asarray(a_src1),
                           np.asarray(a_dst1), np.asarray(b1), np.asarray(W2),
                           np.asarray(a_src2), np.asarray(a_dst2),
                           np.asarray(b2))
    res = run_bass_kernel_spmd(nc, in_maps, list(range(cfg.NCORES)))
    shards = [res.results[c]["emb"] for c in range(cfg.NCORES)]
    return np.concatenate(shards, axis=0)
